# revision 1
# baseline (speedup 1.0000x reference)
"""Trainium2 Bass kernel for nn_ASC_LSTM (per-step LSTM encoder/decoder).

Strategy: data-parallel over batch (32 rows/core x 8 cores). Weights are
replicated, host-scaled by 64 and quantized to fp8 e3m4 (all 16-bit
on-chip surfaces use float16, whose 10-bit mantissa keeps the
recurrent-chain rounding error ~8x below bfloat16), then streamed from
HBM in 1-step chunks (one large DMA per chunk, triple-buffered; the
fine granularity lets the Tile scheduler interleave the decoder weight
stream so the DMA engine runs gapless end-to-end).
Gates are computed transposed ([gate_rows, batch] in PSUM) with all 16
gate chunks of a step accumulated into a single PSUM bank; the per-step
bias is folded in with one K=16 matmul against a one-hot "ones" tensor.
Gate order is [i, f, o, g] so one sigmoid covers chunks 0:12 and one
tanh covers 12:16 (PSUM read + 1/64 descale fused into the activation).
The elu is batched per 8 steps (its Exp needs a different activation
table than sigmoid/tanh); its "-1" is folded into the decoder bias on
the host via row sums of the quantized decoder weights, which is exact
because the skip blend coefficients sum to 1. The sequential skip-blend
chain is re-expressed as a running v-chain (computed as elu blocks
finish) plus 16 independent end corrections u'[4n] = v[n] +
2^-(n+1) * u[60], issued in descending n to match the decoder's
consumption order. Decoder weights (7 rotating buffers) are scheduled
into the stream by the Tile scheduler itself; the remaining ~8us is
fixed startup latency plus the serial tail of the last decoder block,
at the cost model's DMA floor (~360us) for weights-replicated designs.
Next step beyond this floor: timestep-pipeline parallelism across the
8 cores to split weight streaming (needs cross-core remote DMA).
"""
import os
import sys

import numpy as np
import ml_dtypes

sys.path.insert(0, "/opt/trn_rl_repo")

import concourse.bass as bass
import concourse.tile as tile
from concourse import bacc, mybir
from concourse import bass_utils

B, I, H, S, RES = 256, 256, 512, 64, 4
NCORES = 8
BLOC = B // NCORES  # 32
ECH = 1  # encoder steps per weight-DMA chunk
DCH = 4  # decoder idxs per weight-DMA chunk
WSCALE = 64.0
F16 = mybir.dt.float16
F32 = mybir.dt.float32
FP8 = mybir.dt.float8e3
AF = mybir.ActivationFunctionType

_STATE = {}


def _build_module():
    nc = bacc.Bacc(
        "TRN2",
        target_bir_lowering=False,
        debug=False,
        enable_asserts=False,
        num_devices=NCORES,
    )
    wt_d = nc.dram_tensor("wt", [128, S, 6, 16, 128], FP8, kind="ExternalInput").ap()
    wdt_d = nc.dram_tensor("wdt", [128, S, 4, 6, 128], FP8, kind="ExternalInput").ap()
    x_d = nc.dram_tensor("xr", [128, S, 2, BLOC], F16, kind="ExternalInput").ap()
    benc_d = nc.dram_tensor("benc", [16, S, 128], F16, kind="ExternalInput").ap()
    bdec_d = nc.dram_tensor("bdec", [12, S // 2, 128], F16, kind="ExternalInput").ap()
    eones_d = nc.dram_tensor("eones", [16, 16, BLOC], F16, kind="ExternalInput").ap()
    dones_d = nc.dram_tensor("dones", [12, 6, 2, BLOC], F16, kind="ExternalInput").ap()
    out_d = nc.dram_tensor("out", [128, S, 2, BLOC], F16, kind="ExternalOutput").ap()

    inv = 1.0 / WSCALE

    with tile.TileContext(nc) as tc:
        with (
            tc.tile_pool(name="wenc", bufs=3) as wpool,
            tc.tile_pool(name="wdec", bufs=7) as wdpool,
            tc.tile_pool(name="big", bufs=1) as bigpool,
            tc.tile_pool(name="gates", bufs=2) as gpool,
            tc.tile_pool(name="small", bufs=2) as spool,
            tc.tile_pool(name="psum", bufs=4, space="PSUM") as psum,
        ):
            x_sb = bigpool.tile([128, S, 2, BLOC], F16, tag="xsb")
            nc.sync.dma_start(out=x_sb, in_=x_d)
            benc_sb = bigpool.tile([16, S, 128], F16, tag="benc")
            nc.sync.dma_start(out=benc_sb, in_=benc_d)
            eones_sb = bigpool.tile([16, 16, BLOC], F16, tag="eones")
            nc.sync.dma_start(out=eones_sb, in_=eones_d)
            dones_sb = bigpool.tile([12, 6, 2, BLOC], F16, tag="dones")
            nc.sync.dma_start(out=dones_sb, in_=dones_d)
            bdec_sb = bigpool.tile([12, S // 2, 128], F16, tag="bdec")
            nc.sync.dma_start(out=bdec_sb, in_=bdec_d)

            # f16 h history; becomes u = elu(h)+1 in place, then blended.
            hist = bigpool.tile([128, S, 4, BLOC], F16, tag="hist")
            vtile = bigpool.tile([128, S // RES, 4, BLOC], F16, tag="vt")
            out_sb = bigpool.tile([128, S, 2, BLOC], F16, tag="outsb")

            dec_w = {}

            # ---------------- encoder scan ----------------
            # the whole recurrence chain runs in f16 (DVE 2x mode); the next
            # step reads hist[:, t-1] directly, and elu blocks are delayed by
            # one step so they never overwrite a slot the next step still
            # needs
            def elu_block(t0, size, ks):
                blk = hist[:, t0 : t0 + size]
                en_full = spool.tile([128, 8, 4, BLOC], F16, tag="eneg")
                en = en_full[:, :size]
                nc.vector.tensor_scalar_min(en, blk, 0.0)
                nc.vector.tensor_scalar_max(blk, blk, 0.0)
                nc.scalar.activation(out=en, in_=en, func=AF.Exp)
                nc.vector.tensor_add(blk, blk, en)
                # v-chain updates for blend positions now available:
                # v[n] = (u[4n] + v[n-1])/2
                for k in ks:
                    n = k // RES
                    if n == 0:
                        nc.vector.tensor_scalar_mul(vtile[:, 0], hist[:, 0], 0.5)
                    else:
                        nc.vector.tensor_add(vtile[:, n], hist[:, k], vtile[:, n - 1])
                        nc.vector.tensor_scalar_mul(vtile[:, n], vtile[:, n], 0.5)

            for c in range(S // ECH):
                w_sb = wpool.tile([128, ECH, 6, 16, 128], FP8, tag="w")
                nc.sync.dma_start(out=w_sb, in_=wt_d[:, c * ECH : (c + 1) * ECH])
                for i in range(ECH):
                    t = c * ECH + i
                    ps = psum.tile([128, 16, BLOC], F32, tag="ps")
                    nc.tensor.matmul(
                        ps, lhsT=benc_sb[:, t], rhs=eones_sb,
                        start=True, stop=False, skip_group_check=True,
                    )
                    # x-dependent matmuls first: the PE queue is in-order, so
                    # issuing these before the h-matmuls lets the PE work
                    # while the previous step's h is still being produced
                    for m in range(16):
                        for k in range(2):
                            nc.tensor.matmul(
                                ps[:, m], lhsT=w_sb[:, i, k, m], rhs=x_sb[:, t, k],
                                start=False, stop=(t == 0 and k == 1),
                                skip_group_check=True,
                            )
                    if t > 0:
                        # g-gate chunks (12:16) first so the tanh activation
                        # overlaps the remaining h-matmuls
                        for m in (12, 13, 14, 15, 0, 1, 2, 3, 4, 5, 6, 7, 8, 9, 10, 11):
                            for k in range(2, 6):
                                nc.tensor.matmul(
                                    ps[:, m], lhsT=w_sb[:, i, k, m], rhs=hist[:, t - 1, k - 2],
                                    start=False, stop=(k == 5),
                                    skip_group_check=True,
                                )
                    gs = gpool.tile([128, 16, BLOC], F16, tag="gs")
                    nc.scalar.activation(out=gs[:, 12:16], in_=ps[:, 12:16], func=AF.Tanh, scale=inv)
                    nc.scalar.activation(out=gs[:, 0:12], in_=ps[:, 0:12], func=AF.Sigmoid, scale=inv)
                    # c = f*h_prev + i*g ; h = o*tanh(c)
                    cc = spool.tile([128, 4, BLOC], F16, tag="cc")
                    nc.vector.tensor_mul(cc, gs[:, 0:4], gs[:, 12:16])
                    if t > 0:
                        fh = spool.tile([128, 4, BLOC], F16, tag="fh")
                        nc.vector.tensor_mul(fh, gs[:, 4:8], hist[:, t - 1])
                        nc.vector.tensor_add(cc, cc, fh)
                    tct = spool.tile([128, 4, BLOC], F16, tag="tct")
                    nc.scalar.activation(out=tct, in_=cc, func=AF.Tanh)
                    nc.vector.tensor_mul(hist[:, t], tct, gs[:, 8:12])
                    # delayed batched elu: u = relu(h) + exp(min(h,0)), the
                    # -1 is folded into the decoder bias on host
                    if t % 8 == 0 and t > 0:
                        elu_block(t - 8, 8, (t - 8, t - 4))
                    elif t == S - 4:
                        elu_block(S - 8, 4, (S - 8,))
            # finish the elu fine-grained: t=62..63 first so the decoder's
            # first pair (tsrc 63, 62) starts while 60..61 processes
            elu_block(S - 2, 2, ())
            elu_block(S - 4, 2, (S - 4,))

            # ---------------- skip blend end corrections ----------------
            # u'[4n] = v[n] + 2^-(n+1) * u[60]; descending n matches the
            # decoder's consumption order (idx 4j+3 reads t = 60-4j).
            u60 = spool.tile([128, 4, BLOC], F16, tag="u60")
            nc.vector.tensor_copy(out=u60, in_=hist[:, S - RES])
            for n in range(S // RES - 1, -1, -1):
                bc = spool.tile([128, 4, BLOC], F16, tag="bc")
                nc.vector.tensor_scalar_mul(bc, u60, 0.5 ** (n + 1))
                nc.vector.tensor_add(hist[:, n * RES], vtile[:, n], bc)

            # ---------------- decoder (parallel over idx, 2 idx/batch) ----
            rn_prev = None
            for c in range(S // DCH):
                if c in dec_w:
                    wd_sb = dec_w.pop(c)
                else:
                    wd_sb = wdpool.tile([128, DCH, 4, 6, 128], FP8, tag="wd")
                    nc.sync.dma_start(out=wd_sb, in_=wdt_d[:, c * DCH : (c + 1) * DCH])
                for jp in range(DCH // 2):
                    i0 = c * DCH + jp * 2  # idx pair (i0, i0+1)
                    psd = psum.tile([128, 6, 2, BLOC], F32, tag="psd")
                    nc.tensor.matmul(
                        psd, lhsT=bdec_sb[:, i0 // 2], rhs=dones_sb,
                        start=True, stop=False, skip_group_check=True,
                    )
                    for j in range(2):
                        tsrc = S - 1 - (i0 + j)
                        for m in range(6):
                            for k in range(4):
                                nc.tensor.matmul(
                                    psd[:, m, j],
                                    lhsT=wd_sb[:, jp * 2 + j, k, m],
                                    rhs=hist[:, tsrc, k],
                                    start=False, stop=(k == 3),
                                    skip_group_check=True,
                                )
                    gd = gpool.tile([128, 6, 2, BLOC], F16, tag="gd")
                    nc.scalar.activation(out=gd[:, 0:4], in_=psd[:, 0:4], func=AF.Sigmoid, scale=inv)
                    nc.scalar.activation(out=gd[:, 4:6], in_=psd[:, 4:6], func=AF.Tanh, scale=inv)
                    cd = spool.tile([128, 2, 2, BLOC], F16, tag="cd")
                    nc.vector.tensor_mul(cd, gd[:, 0:2], gd[:, 4:6])
                    nc.scalar.activation(out=cd, in_=cd, func=AF.Tanh)
                    nc.vector.tensor_mul(cd, cd, gd[:, 2:4])  # hd, [128, hh, j, b]
                    hdT = cd.transpose([0, 2, 1, 3])  # [128, j, hh, b] view
                    rn = spool.tile([128, 2, 2, BLOC], F16, tag="rn")
                    if i0 % RES == 0:
                        nc.vector.tensor_copy(out=rn[:, 0], in_=hdT[:, 0])
                    else:
                        nc.vector.tensor_add(rn[:, 0], rn_prev[:, 1], hdT[:, 0])
                    nc.vector.tensor_add(rn[:, 1], rn[:, 0], hdT[:, 1])
                    rn_prev = rn
                    nc.scalar.activation(out=out_sb[:, i0 : i0 + 2], in_=rn, func=AF.Tanh)
                if c % 4 == 3 and c < 12:
                    s0 = (c - 3) * DCH
                    nc.sync.dma_start(
                        out=out_d[:, s0 : s0 + 16], in_=out_sb[:, s0 : s0 + 16]
                    )
                elif c in (13, 14):
                    s0 = c * DCH - 4
                    nc.sync.dma_start(
                        out=out_d[:, s0 : s0 + 8], in_=out_sb[:, s0 : s0 + 8]
                    )
                elif c == 15:
                    nc.sync.dma_start(
                        out=out_d[:, 60:64], in_=out_sb[:, 60:64]
                    )
    nc.finalize()
    return nc


def _host_prep(inputs):
    f16 = np.float16
    f8 = ml_dtypes.float8_e3m4
    # encoder: gate order [i, f, o, g]
    eperm = np.r_[0:512, 512:1024, 1536:2048, 1024:1536]
    W_all = np.concatenate([inputs["Wih_enc"], inputs["Whh_enc"]], axis=2)[:, eperm, :] * WSCALE
    # [t, 16m, 128q, 6k, 128p] -> [p, t, k, m, q]
    wt = np.ascontiguousarray(
        W_all.reshape(S, 16, 128, 6, 128).transpose(4, 0, 3, 1, 2)
    ).astype(f8)
    benc = np.ascontiguousarray(
        ((inputs["bih_enc"] + inputs["bhh_enc"])[:, eperm] * WSCALE)
        .reshape(S, 16, 128)
        .transpose(1, 0, 2)
    ).astype(f16)
    eones = np.ascontiguousarray(
        np.repeat(np.eye(16, dtype=np.float32)[:, :, None], BLOC, axis=2)
    ).astype(f16)
    # decoder: gate order [i, o, g]
    dperm = np.r_[0:256, 768:1024, 512:768]
    Wd = inputs["Wih_dec"][:, dperm, :] * WSCALE
    wd8 = np.ascontiguousarray(
        Wd.reshape(S, 6, 128, 4, 128).transpose(4, 0, 3, 1, 2)  # [p,t,k,m,q]
    ).astype(f8)
    # fold elu's "-1" into the bias: subtract row sums of the quantized W
    corr = wd8.astype(np.float32).sum(axis=(0, 2))  # [t, m, q]
    bd = ((inputs["bih_dec"] + inputs["bhh_dec"])[:, dperm] * WSCALE).reshape(S, 6, 128) - corr
    # idx-pair packing: bdec[(m*2+j), pair, q] = bd[2*pair+j, m, q]
    bdec = np.ascontiguousarray(
        bd.reshape(S // 2, 2, 6, 128).transpose(2, 1, 0, 3).reshape(12, S // 2, 128)
    ).astype(f16)
    dones = np.ascontiguousarray(
        np.repeat(
            np.eye(12, dtype=np.float32).reshape(12, 6, 2)[:, :, :, None], BLOC, axis=3
        )
    ).astype(f16)
    xr = np.ascontiguousarray(
        inputs["x"].reshape(B, 2, 128, S).transpose(2, 3, 1, 0)
    ).astype(f16)
    return wt, benc, eones, wd8, bdec, dones, xr


def kernel(**inputs):
    inputs = {k: np.asarray(v) for k, v in inputs.items()}
    if "nc" not in _STATE:
        _STATE["nc"] = _build_module()
    nc = _STATE["nc"]
    wt, benc, eones, wdt, bdec, dones, xr = _host_prep(inputs)
    in_maps = []
    for c in range(NCORES):
        in_maps.append(
            {
                "wt": wt,
                "wdt": wdt,
                "benc": benc,
                "bdec": bdec,
                "eones": eones,
                "dones": dones,
                "xr": np.ascontiguousarray(xr[:, :, :, c * BLOC : (c + 1) * BLOC]),
            }
        )
    res = bass_utils.run_bass_kernel_spmd(
        nc,
        in_maps,
        core_ids=list(range(NCORES)),
        trace=bool(int(os.environ.get("BASS_KERNEL_TRACE", "0"))),
    )
    _STATE["last_results"] = res
    outs = []
    for c in range(NCORES):
        o = np.asarray(res.results[c]["out"]).astype(np.float32)  # [128, S, 2, BLOC]
        outs.append(
            np.ascontiguousarray(
                o.transpose(3, 2, 0, 1).reshape(BLOC, 2 * 128, S)[:, :, ::-1]
            )
        )
    return np.concatenate(outs, axis=0).astype(np.float32)



# revision 26
# speedup vs baseline: 1.0023x; 1.0023x over previous
"""Trainium2 Bass kernel for nn_ASC_LSTM (per-step LSTM encoder/decoder).

Strategy: data-parallel over batch (32 rows/core x 8 cores). Weights are
replicated, host-scaled by 64 and quantized to fp8 e3m4 (all 16-bit
on-chip surfaces use float16, whose 10-bit mantissa keeps the
recurrent-chain rounding error ~8x below bfloat16), then streamed from
HBM in 1-step chunks (one large DMA per chunk, triple-buffered; the
fine granularity lets the Tile scheduler interleave the decoder weight
stream so the DMA engine runs gapless end-to-end).
Gates are computed transposed ([gate_rows, batch] in PSUM) with all 16
gate chunks of a step accumulated into a single PSUM bank; the per-step
bias is folded in with one K=16 matmul against a one-hot "ones" tensor.
Gate order is [i, f, o, g] so one sigmoid covers chunks 0:12 and one
tanh covers 12:16 (PSUM read + 1/64 descale fused into the activation).
The elu is batched per 8 steps (its Exp needs a different activation
table than sigmoid/tanh); its "-1" is folded into the decoder bias on
the host via row sums of the quantized decoder weights, which is exact
because the skip blend coefficients sum to 1. The sequential skip-blend
chain is re-expressed as a running v-chain (computed as elu blocks
finish) plus 16 independent end corrections u'[4n] = v[n] +
2^-(n+1) * u[60], issued in descending n to match the decoder's
consumption order. Decoder weights (7 rotating buffers) are scheduled
into the stream by the Tile scheduler itself; the remaining ~8us is
fixed startup latency plus the serial tail of the last decoder block,
at the cost model's DMA floor (~360us) for weights-replicated designs.
Next step beyond this floor: timestep-pipeline parallelism across the
8 cores to split weight streaming (needs cross-core remote DMA).
"""
import os
import sys

import numpy as np
import ml_dtypes

sys.path.insert(0, "/opt/trn_rl_repo")

import concourse.bass as bass
import concourse.tile as tile
from concourse import bacc, mybir
from concourse import bass_utils

B, I, H, S, RES = 256, 256, 512, 64, 4
NCORES = 8
BLOC = B // NCORES  # 32
ECH = 1  # encoder steps per weight-DMA chunk
DCH = 4  # decoder idxs per weight-DMA chunk
WSCALE = 64.0
F16 = mybir.dt.float16
F32 = mybir.dt.float32
FP8 = mybir.dt.float8e3
AF = mybir.ActivationFunctionType

_STATE = {}


def _build_module():
    nc = bacc.Bacc(
        "TRN2",
        target_bir_lowering=False,
        debug=False,
        enable_asserts=False,
        num_devices=NCORES,
    )
    wt_d = nc.dram_tensor("wt", [128, S, 6, 16, 128], FP8, kind="ExternalInput").ap()
    wdt_d = nc.dram_tensor("wdt", [128, S, 4, 6, 128], FP8, kind="ExternalInput").ap()
    x_d = nc.dram_tensor("xr", [128, S, 2, BLOC], F16, kind="ExternalInput").ap()
    benc_d = nc.dram_tensor("benc", [16, S, 128], F16, kind="ExternalInput").ap()
    bdec_d = nc.dram_tensor("bdec", [12, S // 2, 128], F16, kind="ExternalInput").ap()
    eones_d = nc.dram_tensor("eones", [16, 16, BLOC], F16, kind="ExternalInput").ap()
    dones_d = nc.dram_tensor("dones", [12, 6, 2, BLOC], F16, kind="ExternalInput").ap()
    out_d = nc.dram_tensor("out", [128, S, 2, BLOC], F16, kind="ExternalOutput").ap()

    inv = 1.0 / WSCALE

    with tile.TileContext(nc) as tc:
        with (
            tc.tile_pool(name="wenc", bufs=3) as wpool,
            tc.tile_pool(name="wdec", bufs=7) as wdpool,
            tc.tile_pool(name="big", bufs=1) as bigpool,
            tc.tile_pool(name="gates", bufs=2) as gpool,
            tc.tile_pool(name="small", bufs=2) as spool,
            tc.tile_pool(name="psum", bufs=4, space="PSUM") as psum,
        ):
            x_sb = bigpool.tile([128, S, 2, BLOC], F16, tag="xsb")
            nc.sync.dma_start(out=x_sb, in_=x_d)
            benc_sb = bigpool.tile([16, S, 128], F16, tag="benc")
            nc.sync.dma_start(out=benc_sb, in_=benc_d)
            eones_sb = bigpool.tile([16, 16, BLOC], F16, tag="eones")
            nc.sync.dma_start(out=eones_sb, in_=eones_d)
            dones_sb = bigpool.tile([12, 6, 2, BLOC], F16, tag="dones")
            nc.sync.dma_start(out=dones_sb, in_=dones_d)
            bdec_sb = bigpool.tile([12, S // 2, 128], F16, tag="bdec")
            nc.sync.dma_start(out=bdec_sb, in_=bdec_d)

            # f16 h history; becomes u = elu(h)+1 in place, then blended.
            hist = bigpool.tile([128, S, 4, BLOC], F16, tag="hist")
            vtile = bigpool.tile([128, S // RES, 4, BLOC], F16, tag="vt")
            out_sb = bigpool.tile([128, S, 2, BLOC], F16, tag="outsb")

            dec_w = {}

            # ---------------- encoder scan ----------------
            # the whole recurrence chain runs in f16 (DVE 2x mode); the next
            # step reads hist[:, t-1] directly, and elu blocks are delayed by
            # one step so they never overwrite a slot the next step still
            # needs
            def elu_block(t0, size, ks):
                blk = hist[:, t0 : t0 + size]
                en_full = spool.tile([128, 8, 4, BLOC], F16, tag="eneg")
                en = en_full[:, :size]
                nc.vector.tensor_scalar_min(en, blk, 0.0)
                nc.vector.tensor_scalar_max(blk, blk, 0.0)
                nc.scalar.activation(out=en, in_=en, func=AF.Exp)
                nc.vector.tensor_add(blk, blk, en)
                # v-chain updates for blend positions now available:
                # v[n] = (u[4n] + v[n-1])/2
                for k in ks:
                    n = k // RES
                    if n == 0:
                        nc.vector.tensor_scalar_mul(vtile[:, 0], hist[:, 0], 0.5)
                    else:
                        nc.vector.tensor_add(vtile[:, n], hist[:, k], vtile[:, n - 1])
                        nc.vector.tensor_scalar_mul(vtile[:, n], vtile[:, n], 0.5)

            for c in range(S // ECH):
                w_sb = wpool.tile([128, ECH, 6, 16, 128], FP8, tag="w")
                nc.sync.dma_start(out=w_sb, in_=wt_d[:, c * ECH : (c + 1) * ECH])
                for i in range(ECH):
                    t = c * ECH + i
                    ps = psum.tile([128, 16, BLOC], F32, tag="ps")
                    nc.tensor.matmul(
                        ps, lhsT=benc_sb[:, t], rhs=eones_sb,
                        start=True, stop=False, skip_group_check=True,
                    )
                    # x-dependent matmuls first: the PE queue is in-order, so
                    # issuing these before the h-matmuls lets the PE work
                    # while the previous step's h is still being produced
                    for m in range(16):
                        for k in range(2):
                            nc.tensor.matmul(
                                ps[:, m], lhsT=w_sb[:, i, k, m], rhs=x_sb[:, t, k],
                                start=False, stop=(t == 0 and k == 1),
                                skip_group_check=True,
                            )
                    if t > 0:
                        # g-gate chunks (12:16) first so the tanh activation
                        # overlaps the remaining h-matmuls
                        for m in (12, 13, 14, 15, 0, 1, 2, 3, 4, 5, 6, 7, 8, 9, 10, 11):
                            for k in range(2, 6):
                                nc.tensor.matmul(
                                    ps[:, m], lhsT=w_sb[:, i, k, m], rhs=hist[:, t - 1, k - 2],
                                    start=False, stop=(k == 5),
                                    skip_group_check=True,
                                )
                    gs = gpool.tile([128, 16, BLOC], F16, tag="gs")
                    nc.scalar.activation(out=gs[:, 12:16], in_=ps[:, 12:16], func=AF.Tanh, scale=inv)
                    nc.scalar.activation(out=gs[:, 0:12], in_=ps[:, 0:12], func=AF.Sigmoid, scale=inv)
                    # c = f*h_prev + i*g ; h = o*tanh(c)
                    cc = spool.tile([128, 4, BLOC], F16, tag="cc")
                    nc.vector.tensor_mul(cc, gs[:, 0:4], gs[:, 12:16])
                    if t > 0:
                        fh = spool.tile([128, 4, BLOC], F16, tag="fh")
                        nc.vector.tensor_mul(fh, gs[:, 4:8], hist[:, t - 1])
                        nc.vector.tensor_add(cc, cc, fh)
                    tct = spool.tile([128, 4, BLOC], F16, tag="tct")
                    nc.scalar.activation(out=tct, in_=cc, func=AF.Tanh)
                    nc.vector.tensor_mul(hist[:, t], tct, gs[:, 8:12])
                    # delayed batched elu: u = relu(h) + exp(min(h,0)), the
                    # -1 is folded into the decoder bias on host
                    if t % 8 == 0 and t > 0:
                        elu_block(t - 8, 8, (t - 8, t - 4))
                    elif t == S - 4:
                        elu_block(S - 8, 4, (S - 8,))
            # finish the elu fine-grained: t=62..63 first so the decoder's
            # first pair (tsrc 63, 62) starts while 60..61 processes
            elu_block(S - 2, 2, ())
            elu_block(S - 4, 2, (S - 4,))

            # ---------------- skip blend end corrections ----------------
            # u'[4n] = v[n] + 2^-(n+1) * u[60]; descending n matches the
            # decoder's consumption order (idx 4j+3 reads t = 60-4j).
            u60 = spool.tile([128, 4, BLOC], F16, tag="u60")
            nc.vector.tensor_copy(out=u60, in_=hist[:, S - RES])
            for n in range(S // RES - 1, -1, -1):
                bc = spool.tile([128, 4, BLOC], F16, tag="bc")
                nc.vector.tensor_scalar_mul(bc, u60, 0.5 ** (n + 1))
                nc.vector.tensor_add(hist[:, n * RES], vtile[:, n], bc)

            # ---------------- decoder (parallel over idx, 2 idx/batch) ----
            rn_prev = None
            for c in range(S // DCH):
                wd_sb = wdpool.tile([128, DCH, 4, 6, 128], FP8, tag="wd")
                nc.sync.dma_start(out=wd_sb, in_=wdt_d[:, c * DCH : (c + 1) * DCH])
                for jp in range(DCH // 2):
                    i0 = c * DCH + jp * 2  # idx pair (i0, i0+1)
                    psd = psum.tile([128, 6, 2, BLOC], F32, tag="psd")
                    nc.tensor.matmul(
                        psd, lhsT=bdec_sb[:, i0 // 2], rhs=dones_sb,
                        start=True, stop=False, skip_group_check=True,
                    )
                    for j in range(2):
                        tsrc = S - 1 - (i0 + j)
                        for m in range(6):
                            for k in range(4):
                                nc.tensor.matmul(
                                    psd[:, m, j],
                                    lhsT=wd_sb[:, jp * 2 + j, k, m],
                                    rhs=hist[:, tsrc, k],
                                    start=False, stop=(k == 3),
                                    skip_group_check=True,
                                )
                    gd = gpool.tile([128, 6, 2, BLOC], F16, tag="gd")
                    nc.scalar.activation(out=gd[:, 0:4], in_=psd[:, 0:4], func=AF.Sigmoid, scale=inv)
                    nc.scalar.activation(out=gd[:, 4:6], in_=psd[:, 4:6], func=AF.Tanh, scale=inv)
                    cd = spool.tile([128, 2, 2, BLOC], F16, tag="cd")
                    nc.vector.tensor_mul(cd, gd[:, 0:2], gd[:, 4:6])
                    nc.scalar.activation(out=cd, in_=cd, func=AF.Tanh)
                    nc.vector.tensor_mul(cd, cd, gd[:, 2:4])  # hd, [128, hh, j, b]
                    hdT = cd.transpose([0, 2, 1, 3])  # [128, j, hh, b] view
                    rn = spool.tile([128, 2, 2, BLOC], F16, tag="rn")
                    if i0 % RES == 0:
                        nc.vector.tensor_copy(out=rn[:, 0], in_=hdT[:, 0])
                    else:
                        nc.vector.tensor_add(rn[:, 0], rn_prev[:, 1], hdT[:, 0])
                    nc.vector.tensor_add(rn[:, 1], rn[:, 0], hdT[:, 1])
                    rn_prev = rn
                    nc.scalar.activation(out=out_sb[:, i0 : i0 + 2], in_=rn, func=AF.Tanh)
                # out DMA per 4-step chunk on the gpsimd SWDGE queue: keeps the
                # SP weight stream free of head-of-line blocking on decoder
                # compute, and the DMA device fills its idle slots with these
                s0 = c * DCH
                if c == 15:
                    # split the final chunk: the first pair leaves on gpsimd as
                    # soon as it is done; only the last pair's 2 steps remain
                    # after the final decoder chain, on the SP queue (cheapest
                    # fixed overhead, and the weight stream is finished)
                    nc.gpsimd.dma_start(out=out_d[:, 60:62], in_=out_sb[:, 60:62])
                    nc.sync.dma_start(out=out_d[:, 62:64], in_=out_sb[:, 62:64])
                else:
                    nc.gpsimd.dma_start(
                        out=out_d[:, s0 : s0 + DCH], in_=out_sb[:, s0 : s0 + DCH]
                    )
    nc.finalize()
    return nc


def _host_prep(inputs):
    f16 = np.float16
    f8 = ml_dtypes.float8_e3m4
    # encoder: gate order [i, f, o, g]
    eperm = np.r_[0:512, 512:1024, 1536:2048, 1024:1536]
    W_all = np.concatenate([inputs["Wih_enc"], inputs["Whh_enc"]], axis=2)[:, eperm, :] * WSCALE
    # [t, 16m, 128q, 6k, 128p] -> [p, t, k, m, q]
    wt = np.ascontiguousarray(
        W_all.reshape(S, 16, 128, 6, 128).transpose(4, 0, 3, 1, 2)
    ).astype(f8)
    benc = np.ascontiguousarray(
        ((inputs["bih_enc"] + inputs["bhh_enc"])[:, eperm] * WSCALE)
        .reshape(S, 16, 128)
        .transpose(1, 0, 2)
    ).astype(f16)
    eones = np.ascontiguousarray(
        np.repeat(np.eye(16, dtype=np.float32)[:, :, None], BLOC, axis=2)
    ).astype(f16)
    # decoder: gate order [i, o, g]
    dperm = np.r_[0:256, 768:1024, 512:768]
    Wd = inputs["Wih_dec"][:, dperm, :] * WSCALE
    wd8 = np.ascontiguousarray(
        Wd.reshape(S, 6, 128, 4, 128).transpose(4, 0, 3, 1, 2)  # [p,t,k,m,q]
    ).astype(f8)
    # fold elu's "-1" into the bias: subtract row sums of the quantized W
    corr = wd8.astype(np.float32).sum(axis=(0, 2))  # [t, m, q]
    bd = ((inputs["bih_dec"] + inputs["bhh_dec"])[:, dperm] * WSCALE).reshape(S, 6, 128) - corr
    # idx-pair packing: bdec[(m*2+j), pair, q] = bd[2*pair+j, m, q]
    bdec = np.ascontiguousarray(
        bd.reshape(S // 2, 2, 6, 128).transpose(2, 1, 0, 3).reshape(12, S // 2, 128)
    ).astype(f16)
    dones = np.ascontiguousarray(
        np.repeat(
            np.eye(12, dtype=np.float32).reshape(12, 6, 2)[:, :, :, None], BLOC, axis=3
        )
    ).astype(f16)
    xr = np.ascontiguousarray(
        inputs["x"].reshape(B, 2, 128, S).transpose(2, 3, 1, 0)
    ).astype(f16)
    return wt, benc, eones, wd8, bdec, dones, xr


def kernel(**inputs):
    inputs = {k: np.asarray(v) for k, v in inputs.items()}
    if "nc" not in _STATE:
        _STATE["nc"] = _build_module()
    nc = _STATE["nc"]
    wt, benc, eones, wdt, bdec, dones, xr = _host_prep(inputs)
    in_maps = []
    for c in range(NCORES):
        in_maps.append(
            {
                "wt": wt,
                "wdt": wdt,
                "benc": benc,
                "bdec": bdec,
                "eones": eones,
                "dones": dones,
                "xr": np.ascontiguousarray(xr[:, :, :, c * BLOC : (c + 1) * BLOC]),
            }
        )
    res = bass_utils.run_bass_kernel_spmd(
        nc,
        in_maps,
        core_ids=list(range(NCORES)),
        trace=bool(int(os.environ.get("BASS_KERNEL_TRACE", "0"))),
    )
    _STATE["last_results"] = res
    outs = []
    for c in range(NCORES):
        o = np.asarray(res.results[c]["out"]).astype(np.float32)  # [128, S, 2, BLOC]
        outs.append(
            np.ascontiguousarray(
                o.transpose(3, 2, 0, 1).reshape(BLOC, 2 * 128, S)[:, :, ::-1]
            )
        )
    return np.concatenate(outs, axis=0).astype(np.float32)



# revision 40
# speedup vs baseline: 1.0048x; 1.0026x over previous
"""Trainium2 Bass kernel for nn_ASC_LSTM (per-step LSTM encoder/decoder).

Strategy: data-parallel over batch (32 rows/core x 8 cores). Weights are
replicated, host-scaled by 64 and quantized to fp8 e3m4 (all 16-bit
on-chip surfaces use float16, whose 10-bit mantissa keeps the
recurrent-chain rounding error ~8x below bfloat16), then streamed from
HBM in 1-step chunks (one large DMA per chunk, triple-buffered; the
fine granularity lets the Tile scheduler interleave the decoder weight
stream so the DMA engine runs gapless end-to-end).
Gates are computed transposed ([gate_rows, batch] in PSUM) with all 16
gate chunks of a step accumulated into a single PSUM bank; the per-step
bias is folded in with one K=16 matmul against a one-hot "ones" tensor.
Gate order is [i, f, o, g] so one sigmoid covers chunks 0:12 and one
tanh covers 12:16 (PSUM read + 1/64 descale fused into the activation).
The elu is batched per 8 steps (its Exp needs a different activation
table than sigmoid/tanh); its "-1" is folded into the decoder bias on
the host via row sums of the quantized decoder weights, which is exact
because the skip blend coefficients sum to 1. The sequential skip-blend
chain is re-expressed as a running v-chain (computed as elu blocks
finish) plus 16 independent end corrections u'[4n] = v[n] +
2^-(n+1) * u[60], issued in descending n to match the decoder's
consumption order. Decoder weights (7 rotating buffers) are scheduled
into the stream by the Tile scheduler itself. Output chunks leave per
4 idxs on the gpsimd SWDGE queue (the SP HWDGE queue would head-of-line
block the weight stream on decoder compute); the final 2 idxs go via SP
after the stream is done (cheapest fixed overhead). This sits ~9us over
the per-core DMA floor (126MB replicated weights / 360GB/s = 356.6us):
~2us first-DMA latency, ~5us serial act/DVE chain of the last decoder
block, ~1.5us final sem-prop+drain.
Cross-core sharding (timestep pipeline) was investigated and is not
expressible profitably under the grading cost model: remote-DMA sem
waits deadlock the single-core TimelineSim, register-valued waits
assert (no interp_mem), and collective_compute costs a flat 15us each —
any wavefront/pipeline needs O(P+cores) rendezvous or re-streams
weights per section, always landing at/above the replicated floor.
"""
import os
import sys

import numpy as np
import ml_dtypes

sys.path.insert(0, "/opt/trn_rl_repo")

import concourse.bass as bass
import concourse.tile as tile
from concourse import bacc, mybir
from concourse import bass_utils

B, I, H, S, RES = 256, 256, 512, 64, 4
NCORES = 8
BLOC = B // NCORES  # 32
ECH = 1  # encoder steps per weight-DMA chunk
DCH = 4  # decoder idxs per weight-DMA chunk
WSCALE = 64.0
F16 = mybir.dt.float16
F32 = mybir.dt.float32
FP8 = mybir.dt.float8e3
AF = mybir.ActivationFunctionType

_STATE = {}


def _build_module():
    nc = bacc.Bacc(
        "TRN2",
        target_bir_lowering=False,
        debug=False,
        enable_asserts=False,
        num_devices=NCORES,
    )
    wt_d = nc.dram_tensor("wt", [128, S, 6, 16, 128], FP8, kind="ExternalInput").ap()
    wdt_d = nc.dram_tensor("wdt", [128, S, 4, 6, 128], FP8, kind="ExternalInput").ap()
    x_d = nc.dram_tensor("xr", [128, S, 2, BLOC], F16, kind="ExternalInput").ap()
    benc_d = nc.dram_tensor("benc", [16, S, 128], FP8, kind="ExternalInput").ap()
    bdec_d = nc.dram_tensor("bdec", [12, S // 2, 128], F16, kind="ExternalInput").ap()
    eones_d = nc.dram_tensor("eones", [16, 16, BLOC], FP8, kind="ExternalInput").ap()
    dones_d = nc.dram_tensor("dones", [12, 6, 2, BLOC], FP8, kind="ExternalInput").ap()
    out_d = nc.dram_tensor("out", [128, S, 2, BLOC], F16, kind="ExternalOutput").ap()

    inv = 1.0 / WSCALE

    with tile.TileContext(nc) as tc:
        with (
            tc.tile_pool(name="wenc", bufs=3) as wpool,
            tc.tile_pool(name="wdec", bufs=7) as wdpool,
            tc.tile_pool(name="big", bufs=1) as bigpool,
            tc.tile_pool(name="gates", bufs=2) as gpool,
            tc.tile_pool(name="small", bufs=2) as spool,
            tc.tile_pool(name="psum", bufs=4, space="PSUM") as psum,
        ):
            x_sb = bigpool.tile([128, S, 2, BLOC], F16, tag="xsb")
            nc.sync.dma_start(out=x_sb, in_=x_d)
            benc_sb = bigpool.tile([16, S, 128], FP8, tag="benc")
            nc.sync.dma_start(out=benc_sb, in_=benc_d)
            eones_sb = bigpool.tile([16, 16, BLOC], FP8, tag="eones")
            nc.sync.dma_start(out=eones_sb, in_=eones_d)
            dones_sb = bigpool.tile([12, 6, 2, BLOC], FP8, tag="dones")
            nc.sync.dma_start(out=dones_sb, in_=dones_d)
            bdec_sb = bigpool.tile([12, S // 2, 128], F16, tag="bdec")
            nc.sync.dma_start(out=bdec_sb, in_=bdec_d)

            # f16 h history; becomes u = elu(h)+1 in place, then blended.
            hist = bigpool.tile([128, S, 4, BLOC], F16, tag="hist")
            vtile = bigpool.tile([128, S // RES, 4, BLOC], F16, tag="vt")
            out_sb = bigpool.tile([128, S, 2, BLOC], F16, tag="outsb")

            dec_w = {}

            # ---------------- encoder scan ----------------
            # the whole recurrence chain runs in f16 (DVE 2x mode); the next
            # step reads hist[:, t-1] directly, and elu blocks are delayed by
            # one step so they never overwrite a slot the next step still
            # needs
            def elu_block(t0, size, ks):
                blk = hist[:, t0 : t0 + size]
                en_full = spool.tile([128, 8, 4, BLOC], F16, tag="eneg")
                en = en_full[:, :size]
                nc.vector.tensor_scalar_min(en, blk, 0.0)
                nc.vector.tensor_scalar_max(blk, blk, 0.0)
                nc.scalar.activation(out=en, in_=en, func=AF.Exp)
                nc.vector.tensor_add(blk, blk, en)
                # v-chain updates for blend positions now available:
                # v[n] = (u[4n] + v[n-1])/2
                for k in ks:
                    n = k // RES
                    if n == 0:
                        nc.vector.tensor_scalar_mul(vtile[:, 0], hist[:, 0], 0.5)
                    else:
                        nc.vector.tensor_add(vtile[:, n], hist[:, k], vtile[:, n - 1])
                        nc.vector.tensor_scalar_mul(vtile[:, n], vtile[:, n], 0.5)

            for c in range(S // ECH):
                w_sb = wpool.tile([128, ECH, 6, 16, 128], FP8, tag="w")
                nc.sync.dma_start(out=w_sb, in_=wt_d[:, c * ECH : (c + 1) * ECH])
                for i in range(ECH):
                    t = c * ECH + i
                    ps = psum.tile([128, 16, BLOC], F32, tag="ps")
                    nc.tensor.matmul(
                        ps, lhsT=benc_sb[:, t], rhs=eones_sb,
                        start=True, stop=False, skip_group_check=True,
                    )
                    # x-dependent matmuls first: the PE queue is in-order, so
                    # issuing these before the h-matmuls lets the PE work
                    # while the previous step's h is still being produced
                    for m in range(16):
                        for k in range(2):
                            nc.tensor.matmul(
                                ps[:, m], lhsT=w_sb[:, i, k, m], rhs=x_sb[:, t, k],
                                start=False, stop=(t == 0 and k == 1),
                                skip_group_check=True,
                            )
                    if t > 0:
                        # g-gate chunks (12:16) first so the tanh activation
                        # overlaps the remaining h-matmuls
                        for m in (12, 13, 14, 15, 0, 1, 2, 3, 4, 5, 6, 7, 8, 9, 10, 11):
                            for k in range(2, 6):
                                nc.tensor.matmul(
                                    ps[:, m], lhsT=w_sb[:, i, k, m], rhs=hist[:, t - 1, k - 2],
                                    start=False, stop=(k == 5),
                                    skip_group_check=True,
                                )
                    gs = gpool.tile([128, 16, BLOC], F16, tag="gs")
                    nc.scalar.activation(out=gs[:, 12:16], in_=ps[:, 12:16], func=AF.Tanh, scale=inv)
                    nc.scalar.activation(out=gs[:, 0:12], in_=ps[:, 0:12], func=AF.Sigmoid, scale=inv)
                    # c = f*h_prev + i*g ; h = o*tanh(c)
                    cc = spool.tile([128, 4, BLOC], F16, tag="cc")
                    nc.vector.tensor_mul(cc, gs[:, 0:4], gs[:, 12:16])
                    if t > 0:
                        fh = spool.tile([128, 4, BLOC], F16, tag="fh")
                        nc.vector.tensor_mul(fh, gs[:, 4:8], hist[:, t - 1])
                        nc.vector.tensor_add(cc, cc, fh)
                    tct = spool.tile([128, 4, BLOC], F16, tag="tct")
                    nc.scalar.activation(out=tct, in_=cc, func=AF.Tanh)
                    nc.vector.tensor_mul(hist[:, t], tct, gs[:, 8:12])
                    # delayed batched elu: u = relu(h) + exp(min(h,0)), the
                    # -1 is folded into the decoder bias on host
                    if t % 8 == 0 and t > 0:
                        elu_block(t - 8, 8, (t - 8, t - 4))
                    elif t == S - 4:
                        elu_block(S - 8, 4, (S - 8,))
            # finish the elu fine-grained: t=62..63 first so the decoder's
            # first pair (tsrc 63, 62) starts while 60..61 processes
            elu_block(S - 2, 2, ())
            elu_block(S - 4, 2, (S - 4,))

            # ---------------- skip blend end corrections ----------------
            # u'[4n] = v[n] + 2^-(n+1) * u[60]; descending n matches the
            # decoder's consumption order (idx 4j+3 reads t = 60-4j).
            u60 = spool.tile([128, 4, BLOC], F16, tag="u60")
            nc.vector.tensor_copy(out=u60, in_=hist[:, S - RES])
            for n in range(S // RES - 1, -1, -1):
                bc = spool.tile([128, 4, BLOC], F16, tag="bc")
                nc.vector.tensor_scalar_mul(bc, u60, 0.5 ** (n + 1))
                nc.vector.tensor_add(hist[:, n * RES], vtile[:, n], bc)

            # ---------------- decoder (parallel over idx, 2 idx/batch) ----
            rn_prev = None
            for c in range(S // DCH):
                wd_sb = wdpool.tile([128, DCH, 4, 6, 128], FP8, tag="wd")
                tail = c == 15
                if c >= 14:
                    # two partial loads into one tile: the first pair's weights
                    # land a DMA-slot earlier, so its chain overlaps the second
                    # pair's load
                    s0 = c * DCH
                    nc.sync.dma_start(out=wd_sb[:, 0:2], in_=wdt_d[:, s0 : s0 + 2])
                    nc.sync.dma_start(out=wd_sb[:, 2:4], in_=wdt_d[:, s0 + 2 : s0 + 4])
                else:
                    nc.sync.dma_start(
                        out=wd_sb, in_=wdt_d[:, c * DCH : (c + 1) * DCH]
                    )
                # chunk-batched act path (c<15): one tanh(c) and one output
                # tanh per 4-idx chunk instead of per pair — ~370ns less Act
                # busy per chunk, relieving the Act backlog that gates the
                # decoder tail. The final chunk keeps the per-pair path for
                # minimum latency after the last weight chunk lands.
                gds = []
                tail_hd = []
                if not tail:
                    ccd = spool.tile([128, 2, 2, 2, BLOC], F16, tag="ccd")
                    crn = spool.tile([128, 4, 2, BLOC], F16, tag="crn")
                for jp in range(DCH // 2):
                    i0 = c * DCH + jp * 2  # idx pair (i0, i0+1)
                    psd = psum.tile([128, 6, 2, BLOC], F32, tag="psd")
                    nc.tensor.matmul(
                        psd, lhsT=bdec_sb[:, i0 // 2], rhs=dones_sb,
                        start=True, stop=False, skip_group_check=True,
                    )
                    for j in range(2):
                        tsrc = S - 1 - (i0 + j)
                        for m in range(6):
                            for k in range(4):
                                nc.tensor.matmul(
                                    psd[:, m, j],
                                    lhsT=wd_sb[:, jp * 2 + j, k, m],
                                    rhs=hist[:, tsrc, k],
                                    start=False, stop=(k == 3),
                                    skip_group_check=True,
                                )
                    gd = gpool.tile([128, 6, 2, BLOC], F16, tag="gd")
                    gds.append(gd)
                    # g rows are host-scaled by 2, so one sigmoid covers all 6
                    # chunks; tanh(g) = 2*sig(2g) - 1 via a fused DVE affine.
                    # One Act op instead of two relieves the Act backlog that
                    # gates the decoder tail.
                    nc.scalar.activation(out=gd, in_=psd, func=AF.Sigmoid, scale=inv)
                    nc.vector.tensor_scalar(
                        gd[:, 4:6], gd[:, 4:6], 2.0, -1.0,
                        op0=mybir.AluOpType.mult, op1=mybir.AluOpType.add,
                    )
                    if tail:
                        # per-pair hd chain only; rn/out-tanh of pair 0 is
                        # emitted before pair 1's, but pair 0's OUT tanh is
                        # deferred below so the Act queue order becomes
                        # [sig0, tanhcd0, sig1, tanhcd1, out0, out1] — pair
                        # 1's sigmoid is not stuck behind pair 0's out tanh
                        cd = spool.tile([128, 2, 2, BLOC], F16, tag="cd")
                        nc.vector.tensor_mul(cd, gd[:, 0:2], gd[:, 4:6])
                        nc.scalar.activation(out=cd, in_=cd, func=AF.Tanh)
                        nc.vector.tensor_mul(cd, cd, gd[:, 2:4])  # hd
                        hdT = cd.transpose([0, 2, 1, 3])  # [128, j, hh, b]
                        tail_hd.append(hdT)
                        if jp == 0:
                            rn = spool.tile([128, 2, 2, BLOC], F16, tag="rn")
                            nc.vector.tensor_copy(out=rn[:, 0], in_=hdT[:, 0])
                            nc.vector.tensor_add(rn[:, 1], rn[:, 0], hdT[:, 1])
                            tail_rn = rn
                    else:
                        nc.vector.tensor_mul(ccd[:, jp], gd[:, 0:2], gd[:, 4:6])
                if tail:
                    nc.scalar.activation(out=out_sb[:, 60:62], in_=tail_rn, func=AF.Tanh)
                    nc.gpsimd.dma_start(out=out_d[:, 60:62], in_=out_sb[:, 60:62])
                    rn2 = spool.tile([128, 2, 2, BLOC], F16, tag="rn")
                    nc.vector.tensor_add(rn2[:, 0], tail_rn[:, 1], tail_hd[1][:, 0])
                    nc.vector.tensor_add(rn2[:, 1], rn2[:, 0], tail_hd[1][:, 1])
                    nc.scalar.activation(out=out_sb[:, 62:64], in_=rn2, func=AF.Tanh)
                    nc.sync.dma_start(out=out_d[:, 62:64], in_=out_sb[:, 62:64])
                else:
                    nc.scalar.activation(out=ccd, in_=ccd, func=AF.Tanh)
                    for jp in range(2):
                        nc.vector.tensor_mul(ccd[:, jp], ccd[:, jp], gds[jp][:, 2:4])
                    # cumsum within the RES-block (== this chunk) and one
                    # batched output tanh
                    hdT0 = ccd[:, 0].transpose([0, 2, 1, 3])
                    hdT1 = ccd[:, 1].transpose([0, 2, 1, 3])
                    nc.vector.tensor_copy(out=crn[:, 0], in_=hdT0[:, 0])
                    nc.vector.tensor_add(crn[:, 1], crn[:, 0], hdT0[:, 1])
                    nc.vector.tensor_add(crn[:, 2], crn[:, 1], hdT1[:, 0])
                    nc.vector.tensor_add(crn[:, 3], crn[:, 2], hdT1[:, 1])
                    nc.scalar.activation(
                        out=out_sb[:, c * DCH : (c + 1) * DCH], in_=crn, func=AF.Tanh
                    )
                    # out DMA per 4-step chunk on the gpsimd SWDGE queue:
                    # keeps the SP weight stream free of head-of-line blocking
                    # on decoder compute, and the DMA device fills its idle
                    # slots with these (the tail chunk's outs are inline above)
                    s0 = c * DCH
                    nc.gpsimd.dma_start(
                        out=out_d[:, s0 : s0 + DCH], in_=out_sb[:, s0 : s0 + DCH]
                    )
    nc.finalize()
    return nc


def _host_prep(inputs):
    f16 = np.float16
    f8 = ml_dtypes.float8_e3m4
    # encoder: gate order [i, f, o, g]
    eperm = np.r_[0:512, 512:1024, 1536:2048, 1024:1536]
    W_all = np.concatenate([inputs["Wih_enc"], inputs["Whh_enc"]], axis=2)[:, eperm, :] * WSCALE
    # [t, 16m, 128q, 6k, 128p] -> [p, t, k, m, q]
    wt = np.ascontiguousarray(
        W_all.reshape(S, 16, 128, 6, 128).transpose(4, 0, 3, 1, 2)
    ).astype(f8)
    benc = np.ascontiguousarray(
        ((inputs["bih_enc"] + inputs["bhh_enc"])[:, eperm] * WSCALE)
        .reshape(S, 16, 128)
        .transpose(1, 0, 2)
    ).astype(f8)
    eones = np.ascontiguousarray(
        np.repeat(np.eye(16, dtype=np.float32)[:, :, None], BLOC, axis=2)
    ).astype(f8)
    # decoder: gate order [i, o, g]; g rows scaled by 2 so the device can use
    # a single sigmoid table: tanh(g) = 2*sigmoid(2g) - 1
    dperm = np.r_[0:256, 768:1024, 512:768]
    Wd = inputs["Wih_dec"][:, dperm, :] * WSCALE
    Wd[:, 512:768, :] *= 2.0
    wd8 = np.ascontiguousarray(
        Wd.reshape(S, 6, 128, 4, 128).transpose(4, 0, 3, 1, 2)  # [p,t,k,m,q]
    ).astype(f8)
    # fold elu's "-1" into the bias: subtract row sums of the quantized W
    corr = wd8.astype(np.float32).sum(axis=(0, 2))  # [t, m, q]
    braw = (inputs["bih_dec"] + inputs["bhh_dec"])[:, dperm] * WSCALE
    braw[:, 512:768] *= 2.0  # match the g-row scaling
    bd = braw.reshape(S, 6, 128) - corr
    # idx-pair packing: bdec[(m*2+j), pair, q] = bd[2*pair+j, m, q]
    bdec = np.ascontiguousarray(
        bd.reshape(S // 2, 2, 6, 128).transpose(2, 1, 0, 3).reshape(12, S // 2, 128)
    ).astype(f16)
    dones = np.ascontiguousarray(
        np.repeat(
            np.eye(12, dtype=np.float32).reshape(12, 6, 2)[:, :, :, None], BLOC, axis=3
        )
    ).astype(f8)
    xr = np.ascontiguousarray(
        inputs["x"].reshape(B, 2, 128, S).transpose(2, 3, 1, 0)
    ).astype(f16)
    return wt, benc, eones, wd8, bdec, dones, xr


def kernel(**inputs):
    inputs = {k: np.asarray(v) for k, v in inputs.items()}
    if "nc" not in _STATE:
        _STATE["nc"] = _build_module()
    nc = _STATE["nc"]
    wt, benc, eones, wdt, bdec, dones, xr = _host_prep(inputs)
    in_maps = []
    for c in range(NCORES):
        in_maps.append(
            {
                "wt": wt,
                "wdt": wdt,
                "benc": benc,
                "bdec": bdec,
                "eones": eones,
                "dones": dones,
                "xr": np.ascontiguousarray(xr[:, :, :, c * BLOC : (c + 1) * BLOC]),
            }
        )
    res = bass_utils.run_bass_kernel_spmd(
        nc,
        in_maps,
        core_ids=list(range(NCORES)),
        trace=bool(int(os.environ.get("BASS_KERNEL_TRACE", "0"))),
    )
    _STATE["last_results"] = res
    outs = []
    for c in range(NCORES):
        o = np.asarray(res.results[c]["out"]).astype(np.float32)  # [128, S, 2, BLOC]
        outs.append(
            np.ascontiguousarray(
                o.transpose(3, 2, 0, 1).reshape(BLOC, 2 * 128, S)[:, :, ::-1]
            )
        )
    return np.concatenate(outs, axis=0).astype(np.float32)



# revision 41
# speedup vs baseline: 1.0059x; 1.0011x over previous
"""Trainium2 Bass kernel for nn_ASC_LSTM (per-step LSTM encoder/decoder).

Strategy: data-parallel over batch (32 rows/core x 8 cores). Weights are
replicated, host-scaled by 64 and quantized to fp8 e3m4 (all 16-bit
on-chip surfaces use float16, whose 10-bit mantissa keeps the
recurrent-chain rounding error ~8x below bfloat16), then streamed from
HBM in 1-step chunks (one large DMA per chunk, triple-buffered; the
fine granularity lets the Tile scheduler interleave the decoder weight
stream so the DMA engine runs gapless end-to-end).
Gates are computed transposed ([gate_rows, batch] in PSUM) with all 16
gate chunks of a step accumulated into a single PSUM bank; the per-step
bias is folded in with one K=16 matmul against a one-hot "ones" tensor.
Gate order is [i, f, o, g] so one sigmoid covers chunks 0:12 and one
tanh covers 12:16 (PSUM read + 1/64 descale fused into the activation).
The elu is batched per 8 steps (its Exp needs a different activation
table than sigmoid/tanh); its "-1" is folded into the decoder bias on
the host via row sums of the quantized decoder weights, which is exact
because the skip blend coefficients sum to 1. The sequential skip-blend
chain is re-expressed as a running v-chain (computed as elu blocks
finish) plus 16 independent end corrections u'[4n] = v[n] +
2^-(n+1) * u[60], issued in descending n to match the decoder's
consumption order. Decoder weights (7 rotating buffers) are scheduled
into the stream by the Tile scheduler itself; the last two 4-idx chunks
load as pair-granular partial DMAs so the tail compute starts sooner.
Decoder g-gate rows are host-scaled by 2 so a single sigmoid serves all
gates (tanh(g) = 2*sig(2g)-1 via one fused DVE affine), and tanh(c)/
output tanh are batched per 4-idx chunk — both cut the Act-engine
backlog that gates the decoder tail. Biases and one-hot tensors ship as
fp8 (bias quant error enters gates once, un-amplified; one-hots are
exact — bdec stays f16 for the folded elu correction's range). Output
chunks leave per 4 idxs on the gpsimd SWDGE queue (the SP HWDGE queue
would head-of-line block the weight stream on decoder compute); the
final 2 idxs go via SP after the stream is done. This sits ~8us over
the per-core DMA floor (126MB replicated weights / 360GB/s = 356.4us):
~2us first-DMA latency, ~4us serial act/DVE chain of the last decoder
block, ~1.5us final sem-prop+drain.
Cross-core sharding (timestep pipeline) was investigated and is not
expressible profitably under the grading cost model: remote-DMA sem
waits deadlock the single-core TimelineSim, register-valued waits
assert (no interp_mem), and collective_compute costs a flat 15us each —
any wavefront/pipeline needs O(P+cores) rendezvous or re-streams
weights per section, always landing at/above the replicated floor.
"""
import os
import sys

import numpy as np
import ml_dtypes

sys.path.insert(0, "/opt/trn_rl_repo")

import concourse.bass as bass
import concourse.tile as tile
from concourse import bacc, mybir
from concourse import bass_utils

B, I, H, S, RES = 256, 256, 512, 64, 4
NCORES = 8
BLOC = B // NCORES  # 32
ECH = 1  # encoder steps per weight-DMA chunk
DCH = 4  # decoder idxs per weight-DMA chunk
WSCALE = 64.0
F16 = mybir.dt.float16
F32 = mybir.dt.float32
FP8 = mybir.dt.float8e3
AF = mybir.ActivationFunctionType

_STATE = {}


def _build_module():
    nc = bacc.Bacc(
        "TRN2",
        target_bir_lowering=False,
        debug=False,
        enable_asserts=False,
        num_devices=NCORES,
    )
    wt_d = nc.dram_tensor("wt", [128, S, 6, 16, 128], FP8, kind="ExternalInput").ap()
    wdt_d = nc.dram_tensor("wdt", [128, S, 4, 6, 128], FP8, kind="ExternalInput").ap()
    x_d = nc.dram_tensor("xr", [128, S, 2, BLOC], F16, kind="ExternalInput").ap()
    benc_d = nc.dram_tensor("benc", [16, S, 128], FP8, kind="ExternalInput").ap()
    bdec_d = nc.dram_tensor("bdec", [12, S // 2, 128], F16, kind="ExternalInput").ap()
    eones_d = nc.dram_tensor("eones", [16, 16, BLOC], FP8, kind="ExternalInput").ap()
    dones_d = nc.dram_tensor("dones", [12, 6, 2, BLOC], FP8, kind="ExternalInput").ap()
    out_d = nc.dram_tensor("out", [128, S, 2, BLOC], F16, kind="ExternalOutput").ap()

    inv = 1.0 / WSCALE

    with tile.TileContext(nc) as tc:
        with (
            tc.tile_pool(name="wenc", bufs=3) as wpool,
            tc.tile_pool(name="wdec", bufs=7) as wdpool,
            tc.tile_pool(name="big", bufs=1) as bigpool,
            tc.tile_pool(name="gates", bufs=2) as gpool,
            tc.tile_pool(name="small", bufs=2) as spool,
            tc.tile_pool(name="psum", bufs=4, space="PSUM") as psum,
        ):
            x_sb = bigpool.tile([128, S, 2, BLOC], F16, tag="xsb")
            nc.sync.dma_start(out=x_sb, in_=x_d)
            benc_sb = bigpool.tile([16, S, 128], FP8, tag="benc")
            nc.sync.dma_start(out=benc_sb, in_=benc_d)
            eones_sb = bigpool.tile([16, 16, BLOC], FP8, tag="eones")
            nc.sync.dma_start(out=eones_sb, in_=eones_d)
            dones_sb = bigpool.tile([12, 6, 2, BLOC], FP8, tag="dones")
            nc.sync.dma_start(out=dones_sb, in_=dones_d)
            bdec_sb = bigpool.tile([12, S // 2, 128], F16, tag="bdec")
            nc.sync.dma_start(out=bdec_sb, in_=bdec_d)

            # f16 h history; becomes u = elu(h)+1 in place, then blended.
            hist = bigpool.tile([128, S, 4, BLOC], F16, tag="hist")
            vtile = bigpool.tile([128, S // RES, 4, BLOC], F16, tag="vt")
            out_sb = bigpool.tile([128, S, 2, BLOC], F16, tag="outsb")

            dec_w = {}

            # ---------------- encoder scan ----------------
            # the whole recurrence chain runs in f16 (DVE 2x mode); the next
            # step reads hist[:, t-1] directly, and elu blocks are delayed by
            # one step so they never overwrite a slot the next step still
            # needs
            def elu_block(t0, size, ks):
                blk = hist[:, t0 : t0 + size]
                en_full = spool.tile([128, 8, 4, BLOC], F16, tag="eneg")
                en = en_full[:, :size]
                nc.vector.tensor_scalar_min(en, blk, 0.0)
                nc.vector.tensor_scalar_max(blk, blk, 0.0)
                nc.scalar.activation(out=en, in_=en, func=AF.Exp)
                nc.vector.tensor_add(blk, blk, en)
                # v-chain updates for blend positions now available:
                # v[n] = (u[4n] + v[n-1])/2
                for k in ks:
                    n = k // RES
                    if n == 0:
                        nc.vector.tensor_scalar_mul(vtile[:, 0], hist[:, 0], 0.5)
                    else:
                        nc.vector.tensor_add(vtile[:, n], hist[:, k], vtile[:, n - 1])
                        nc.vector.tensor_scalar_mul(vtile[:, n], vtile[:, n], 0.5)

            for c in range(S // ECH):
                w_sb = wpool.tile([128, ECH, 6, 16, 128], FP8, tag="w")
                nc.sync.dma_start(out=w_sb, in_=wt_d[:, c * ECH : (c + 1) * ECH])
                for i in range(ECH):
                    t = c * ECH + i
                    ps = psum.tile([128, 16, BLOC], F32, tag="ps")
                    nc.tensor.matmul(
                        ps, lhsT=benc_sb[:, t], rhs=eones_sb,
                        start=True, stop=False, skip_group_check=True,
                    )
                    # x-dependent matmuls first: the PE queue is in-order, so
                    # issuing these before the h-matmuls lets the PE work
                    # while the previous step's h is still being produced
                    for m in range(16):
                        for k in range(2):
                            nc.tensor.matmul(
                                ps[:, m], lhsT=w_sb[:, i, k, m], rhs=x_sb[:, t, k],
                                start=False, stop=(t == 0 and k == 1),
                                skip_group_check=True,
                            )
                    if t > 0:
                        # g-gate chunks (12:16) first so the tanh activation
                        # overlaps the remaining h-matmuls
                        for m in (12, 13, 14, 15, 0, 1, 2, 3, 4, 5, 6, 7, 8, 9, 10, 11):
                            for k in range(2, 6):
                                nc.tensor.matmul(
                                    ps[:, m], lhsT=w_sb[:, i, k, m], rhs=hist[:, t - 1, k - 2],
                                    start=False, stop=(k == 5),
                                    skip_group_check=True,
                                )
                    gs = gpool.tile([128, 16, BLOC], F16, tag="gs")
                    nc.scalar.activation(out=gs[:, 12:16], in_=ps[:, 12:16], func=AF.Tanh, scale=inv)
                    nc.scalar.activation(out=gs[:, 0:12], in_=ps[:, 0:12], func=AF.Sigmoid, scale=inv)
                    # c = f*h_prev + i*g ; h = o*tanh(c)
                    cc = spool.tile([128, 4, BLOC], F16, tag="cc")
                    nc.vector.tensor_mul(cc, gs[:, 0:4], gs[:, 12:16])
                    if t > 0:
                        fh = spool.tile([128, 4, BLOC], F16, tag="fh")
                        nc.vector.tensor_mul(fh, gs[:, 4:8], hist[:, t - 1])
                        nc.vector.tensor_add(cc, cc, fh)
                    tct = spool.tile([128, 4, BLOC], F16, tag="tct")
                    nc.scalar.activation(out=tct, in_=cc, func=AF.Tanh)
                    nc.vector.tensor_mul(hist[:, t], tct, gs[:, 8:12])
                    # delayed batched elu: u = relu(h) + exp(min(h,0)), the
                    # -1 is folded into the decoder bias on host
                    if t % 8 == 0 and t > 0:
                        elu_block(t - 8, 8, (t - 8, t - 4))
                    elif t == S - 4:
                        elu_block(S - 8, 4, (S - 8,))
            # finish the elu fine-grained: t=62..63 first so the decoder's
            # first pair (tsrc 63, 62) starts while 60..61 processes
            elu_block(S - 2, 2, ())
            elu_block(S - 4, 2, (S - 4,))

            # ---------------- skip blend end corrections ----------------
            # u'[4n] = v[n] + 2^-(n+1) * u[60]; descending n matches the
            # decoder's consumption order (idx 4j+3 reads t = 60-4j).
            u60 = spool.tile([128, 4, BLOC], F16, tag="u60")
            nc.vector.tensor_copy(out=u60, in_=hist[:, S - RES])
            for n in range(S // RES - 1, -1, -1):
                bc = spool.tile([128, 4, BLOC], F16, tag="bc")
                nc.vector.tensor_scalar_mul(bc, u60, 0.5 ** (n + 1))
                nc.vector.tensor_add(hist[:, n * RES], vtile[:, n], bc)

            # ---------------- decoder (parallel over idx, 2 idx/batch) ----
            rn_prev = None
            for c in range(S // DCH):
                wd_sb = wdpool.tile([128, DCH, 4, 6, 128], FP8, tag="wd")
                tail = c == 15
                if c >= 14:
                    # two partial loads into one tile: the first pair's weights
                    # land a DMA-slot earlier, so its chain overlaps the second
                    # pair's load
                    s0 = c * DCH
                    nc.sync.dma_start(out=wd_sb[:, 0:2], in_=wdt_d[:, s0 : s0 + 2])
                    nc.sync.dma_start(out=wd_sb[:, 2:4], in_=wdt_d[:, s0 + 2 : s0 + 4])
                else:
                    nc.sync.dma_start(
                        out=wd_sb, in_=wdt_d[:, c * DCH : (c + 1) * DCH]
                    )
                # chunk-batched act path (c<15): one tanh(c) and one output
                # tanh per 4-idx chunk instead of per pair — ~370ns less Act
                # busy per chunk, relieving the Act backlog that gates the
                # decoder tail. The final chunk keeps the per-pair path for
                # minimum latency after the last weight chunk lands.
                gds = []
                tail_hd = []
                if not tail:
                    ccd = spool.tile([128, 2, 2, 2, BLOC], F16, tag="ccd")
                    crn = spool.tile([128, 4, 2, BLOC], F16, tag="crn")
                for jp in range(DCH // 2):
                    i0 = c * DCH + jp * 2  # idx pair (i0, i0+1)
                    psd = psum.tile([128, 6, 2, BLOC], F32, tag="psd")
                    nc.tensor.matmul(
                        psd, lhsT=bdec_sb[:, i0 // 2], rhs=dones_sb,
                        start=True, stop=False, skip_group_check=True,
                    )
                    for j in range(2):
                        tsrc = S - 1 - (i0 + j)
                        for m in range(6):
                            for k in range(4):
                                nc.tensor.matmul(
                                    psd[:, m, j],
                                    lhsT=wd_sb[:, jp * 2 + j, k, m],
                                    rhs=hist[:, tsrc, k],
                                    start=False, stop=(k == 3),
                                    skip_group_check=True,
                                )
                    gd = gpool.tile([128, 6, 2, BLOC], F16, tag="gd")
                    gds.append(gd)
                    # g rows are host-scaled by 2, so one sigmoid covers all 6
                    # chunks; tanh(g) = 2*sig(2g) - 1 via a fused DVE affine.
                    # One Act op instead of two relieves the Act backlog that
                    # gates the decoder tail.
                    nc.scalar.activation(out=gd, in_=psd, func=AF.Sigmoid, scale=inv)
                    nc.vector.tensor_scalar(
                        gd[:, 4:6], gd[:, 4:6], 2.0, -1.0,
                        op0=mybir.AluOpType.mult, op1=mybir.AluOpType.add,
                    )
                    if tail:
                        # per-pair hd chain only; rn/out-tanh of pair 0 is
                        # emitted before pair 1's, but pair 0's OUT tanh is
                        # deferred below so the Act queue order becomes
                        # [sig0, tanhcd0, sig1, tanhcd1, out0, out1] — pair
                        # 1's sigmoid is not stuck behind pair 0's out tanh
                        cd = spool.tile([128, 2, 2, BLOC], F16, tag="cd")
                        nc.vector.tensor_mul(cd, gd[:, 0:2], gd[:, 4:6])
                        nc.scalar.activation(out=cd, in_=cd, func=AF.Tanh)
                        nc.vector.tensor_mul(cd, cd, gd[:, 2:4])  # hd
                        hdT = cd.transpose([0, 2, 1, 3])  # [128, j, hh, b]
                        tail_hd.append(hdT)
                        if jp == 0:
                            rn = spool.tile([128, 2, 2, BLOC], F16, tag="rn")
                            nc.vector.tensor_copy(out=rn[:, 0], in_=hdT[:, 0])
                            nc.vector.tensor_add(rn[:, 1], rn[:, 0], hdT[:, 1])
                            tail_rn = rn
                    else:
                        nc.vector.tensor_mul(ccd[:, jp], gd[:, 0:2], gd[:, 4:6])
                if tail:
                    nc.scalar.activation(out=out_sb[:, 60:62], in_=tail_rn, func=AF.Tanh)
                    nc.gpsimd.dma_start(out=out_d[:, 60:62], in_=out_sb[:, 60:62])
                    rn2 = spool.tile([128, 2, 2, BLOC], F16, tag="rn")
                    nc.vector.tensor_add(rn2[:, 0], tail_rn[:, 1], tail_hd[1][:, 0])
                    nc.vector.tensor_add(rn2[:, 1], rn2[:, 0], tail_hd[1][:, 1])
                    nc.scalar.activation(out=out_sb[:, 62:64], in_=rn2, func=AF.Tanh)
                    nc.sync.dma_start(out=out_d[:, 62:64], in_=out_sb[:, 62:64])
                else:
                    nc.scalar.activation(out=ccd, in_=ccd, func=AF.Tanh)
                    for jp in range(2):
                        nc.vector.tensor_mul(ccd[:, jp], ccd[:, jp], gds[jp][:, 2:4])
                    # cumsum within the RES-block (== this chunk) and one
                    # batched output tanh
                    hdT0 = ccd[:, 0].transpose([0, 2, 1, 3])
                    hdT1 = ccd[:, 1].transpose([0, 2, 1, 3])
                    nc.vector.tensor_copy(out=crn[:, 0], in_=hdT0[:, 0])
                    nc.vector.tensor_add(crn[:, 1], crn[:, 0], hdT0[:, 1])
                    nc.vector.tensor_add(crn[:, 2], crn[:, 1], hdT1[:, 0])
                    nc.vector.tensor_add(crn[:, 3], crn[:, 2], hdT1[:, 1])
                    nc.scalar.activation(
                        out=out_sb[:, c * DCH : (c + 1) * DCH], in_=crn, func=AF.Tanh
                    )
                    # out DMA per 4-step chunk on the gpsimd SWDGE queue:
                    # keeps the SP weight stream free of head-of-line blocking
                    # on decoder compute, and the DMA device fills its idle
                    # slots with these (the tail chunk's outs are inline above)
                    s0 = c * DCH
                    nc.gpsimd.dma_start(
                        out=out_d[:, s0 : s0 + DCH], in_=out_sb[:, s0 : s0 + DCH]
                    )
    nc.finalize()
    return nc


def _host_prep(inputs):
    f16 = np.float16
    f8 = ml_dtypes.float8_e3m4
    # encoder: gate order [i, f, o, g]
    eperm = np.r_[0:512, 512:1024, 1536:2048, 1024:1536]
    W_all = np.concatenate([inputs["Wih_enc"], inputs["Whh_enc"]], axis=2)[:, eperm, :] * WSCALE
    # [t, 16m, 128q, 6k, 128p] -> [p, t, k, m, q]
    wt = np.ascontiguousarray(
        W_all.reshape(S, 16, 128, 6, 128).transpose(4, 0, 3, 1, 2)
    ).astype(f8)
    benc = np.ascontiguousarray(
        ((inputs["bih_enc"] + inputs["bhh_enc"])[:, eperm] * WSCALE)
        .reshape(S, 16, 128)
        .transpose(1, 0, 2)
    ).astype(f8)
    eones = np.ascontiguousarray(
        np.repeat(np.eye(16, dtype=np.float32)[:, :, None], BLOC, axis=2)
    ).astype(f8)
    # decoder: gate order [i, o, g]; g rows scaled by 2 so the device can use
    # a single sigmoid table: tanh(g) = 2*sigmoid(2g) - 1
    dperm = np.r_[0:256, 768:1024, 512:768]
    Wd = inputs["Wih_dec"][:, dperm, :] * WSCALE
    Wd[:, 512:768, :] *= 2.0
    wd8 = np.ascontiguousarray(
        Wd.reshape(S, 6, 128, 4, 128).transpose(4, 0, 3, 1, 2)  # [p,t,k,m,q]
    ).astype(f8)
    # fold elu's "-1" into the bias: subtract row sums of the quantized W
    corr = wd8.astype(np.float32).sum(axis=(0, 2))  # [t, m, q]
    braw = (inputs["bih_dec"] + inputs["bhh_dec"])[:, dperm] * WSCALE
    braw[:, 512:768] *= 2.0  # match the g-row scaling
    bd = braw.reshape(S, 6, 128) - corr
    # idx-pair packing: bdec[(m*2+j), pair, q] = bd[2*pair+j, m, q]
    bdec = np.ascontiguousarray(
        bd.reshape(S // 2, 2, 6, 128).transpose(2, 1, 0, 3).reshape(12, S // 2, 128)
    ).astype(f16)
    dones = np.ascontiguousarray(
        np.repeat(
            np.eye(12, dtype=np.float32).reshape(12, 6, 2)[:, :, :, None], BLOC, axis=3
        )
    ).astype(f8)
    xr = np.ascontiguousarray(
        inputs["x"].reshape(B, 2, 128, S).transpose(2, 3, 1, 0)
    ).astype(f16)
    return wt, benc, eones, wd8, bdec, dones, xr


def kernel(**inputs):
    inputs = {k: np.asarray(v) for k, v in inputs.items()}
    if "nc" not in _STATE:
        _STATE["nc"] = _build_module()
    nc = _STATE["nc"]
    wt, benc, eones, wdt, bdec, dones, xr = _host_prep(inputs)
    in_maps = []
    for c in range(NCORES):
        in_maps.append(
            {
                "wt": wt,
                "wdt": wdt,
                "benc": benc,
                "bdec": bdec,
                "eones": eones,
                "dones": dones,
                "xr": np.ascontiguousarray(xr[:, :, :, c * BLOC : (c + 1) * BLOC]),
            }
        )
    res = bass_utils.run_bass_kernel_spmd(
        nc,
        in_maps,
        core_ids=list(range(NCORES)),
        trace=bool(int(os.environ.get("BASS_KERNEL_TRACE", "0"))),
    )
    _STATE["last_results"] = res
    outs = []
    for c in range(NCORES):
        o = np.asarray(res.results[c]["out"]).astype(np.float32)  # [128, S, 2, BLOC]
        outs.append(
            np.ascontiguousarray(
                o.transpose(3, 2, 0, 1).reshape(BLOC, 2 * 128, S)[:, :, ::-1]
            )
        )
    return np.concatenate(outs, axis=0).astype(np.float32)



# revision 47
# speedup vs baseline: 1.0071x; 1.0012x over previous
"""Trainium2 Bass kernel for nn_ASC_LSTM (per-step LSTM encoder/decoder).

Strategy: data-parallel over batch (32 rows/core x 8 cores). Weights are
replicated, host-scaled by 64 and quantized to fp8 e3m4 (all 16-bit
on-chip surfaces use float16, whose 10-bit mantissa keeps the
recurrent-chain rounding error ~8x below bfloat16), then streamed from
HBM in 1-step chunks (one large DMA per chunk, triple-buffered; the
fine granularity lets the Tile scheduler interleave the decoder weight
stream so the DMA engine runs gapless end-to-end).
Gates are computed transposed ([gate_rows, batch] in PSUM) with all 16
gate chunks of a step accumulated into a single PSUM bank; the per-step
bias is folded in with one K=16 matmul against a one-hot "ones" tensor.
Gate order is [i, f, o, g] so one sigmoid covers chunks 0:12 and one
tanh covers 12:16 (PSUM read + 1/64 descale fused into the activation).
The elu is batched per 8 steps (its Exp needs a different activation
table than sigmoid/tanh); its "-1" is folded into the decoder bias on
the host via row sums of the quantized decoder weights, which is exact
because the skip blend coefficients sum to 1. The sequential skip-blend
chain is re-expressed as a running v-chain (computed as elu blocks
finish) plus 16 independent end corrections u'[4n] = v[n] +
2^-(n+1) * u[60], issued in descending n to match the decoder's
consumption order. Decoder weights (7 rotating buffers) are scheduled
into the stream by the Tile scheduler itself; the last two 4-idx chunks
load as pair-granular partial DMAs so the tail compute starts sooner.
Decoder g-gate rows are host-scaled by 2 so a single sigmoid serves all
gates (tanh(g) = 2*sig(2g)-1 via one fused DVE affine), and tanh(c)/
output tanh are batched per 4-idx chunk — both cut the Act-engine
backlog that gates the decoder tail. Biases and one-hot tensors ship as
fp8 (bias quant error enters gates once, un-amplified; one-hots are
exact — bdec stays f16 for the folded elu correction's range). Output
chunks leave per 4 idxs on the gpsimd SWDGE queue (the SP HWDGE queue
would head-of-line block the weight stream on decoder compute); the
final 2 idxs go via SP after the stream is done. This sits ~8us over
the per-core DMA floor (126MB replicated weights / 360GB/s = 356.4us),
all of it framework-fixed latency: ~2.0us first-DMA start (preamble
memsets + entry barrier + HWDGE/DGE pipeline), ~3.1us last-chunk chain
(900ns DMA sem-prop + matmuls + 7 serial Act/DVE hops at ~200ns
propagation each), ~3.1us epilogue (HWDGE 625 + DGE 650 + transfer +
900ns sem-prop + exit drains). Reordering the stream cannot shorten
the tail: every candidate last-landing chunk has an equal-or-longer
consumer chain. The one remaining ~1us lever is a scatter-add
prep/trigger for the final output (skips HWDGE+DGE), blocked by Tile's
exit-drain accounting: prepare_only requires a private completion sem,
which diverts the DMASW-lane increments the drain expects.
Cross-core sharding (timestep pipeline) was investigated and is not
expressible profitably under the grading cost model: remote-DMA sem
waits deadlock the single-core TimelineSim, register-valued waits
assert (no interp_mem), and collective_compute costs a flat 15us each —
any wavefront/pipeline needs O(P+cores) rendezvous or re-streams
weights per section, always landing at/above the replicated floor.
"""
import os
import sys

import numpy as np
import ml_dtypes

sys.path.insert(0, "/opt/trn_rl_repo")

import concourse.bass as bass
import concourse.tile as tile
from concourse import bacc, mybir
from concourse import bass_utils

B, I, H, S, RES = 256, 256, 512, 64, 4
NCORES = 8
BLOC = B // NCORES  # 32
ECH = 1  # encoder steps per weight-DMA chunk
DCH = 4  # decoder idxs per weight-DMA chunk
WSCALE = 64.0
F16 = mybir.dt.float16
F32 = mybir.dt.float32
FP8 = mybir.dt.float8e3
AF = mybir.ActivationFunctionType

_STATE = {}


def _build_module():
    nc = bacc.Bacc(
        "TRN2",
        target_bir_lowering=False,
        debug=False,
        enable_asserts=False,
        num_devices=NCORES,
    )
    wt_d = nc.dram_tensor("wt", [128, S, 6, 16, 128], FP8, kind="ExternalInput").ap()
    wdt_d = nc.dram_tensor("wdt", [128, S, 4, 6, 128], FP8, kind="ExternalInput").ap()
    x_d = nc.dram_tensor("xr", [128, S, 2, BLOC], FP8, kind="ExternalInput").ap()
    benc_d = nc.dram_tensor("benc", [16, S, 128], FP8, kind="ExternalInput").ap()
    bdec_d = nc.dram_tensor("bdec", [12, S // 2, 128], F16, kind="ExternalInput").ap()
    eones_d = nc.dram_tensor("eones", [16, 16, BLOC], FP8, kind="ExternalInput").ap()
    dones_d = nc.dram_tensor("dones", [12, 6, 2, BLOC], FP8, kind="ExternalInput").ap()
    out_d = nc.dram_tensor("out", [128, S, 2, BLOC], F16, kind="ExternalOutput").ap()

    inv = 1.0 / WSCALE

    with tile.TileContext(nc) as tc:
        with (
            tc.tile_pool(name="wenc", bufs=3) as wpool,
            tc.tile_pool(name="wdec", bufs=7) as wdpool,
            tc.tile_pool(name="big", bufs=1) as bigpool,
            tc.tile_pool(name="gates", bufs=2) as gpool,
            tc.tile_pool(name="small", bufs=2) as spool,
            tc.tile_pool(name="psum", bufs=4, space="PSUM") as psum,
        ):
            x_sb = bigpool.tile([128, S, 2, BLOC], FP8, tag="xsb")
            nc.sync.dma_start(out=x_sb, in_=x_d)
            benc_sb = bigpool.tile([16, S, 128], FP8, tag="benc")
            nc.sync.dma_start(out=benc_sb, in_=benc_d)
            eones_sb = bigpool.tile([16, 16, BLOC], FP8, tag="eones")
            nc.sync.dma_start(out=eones_sb, in_=eones_d)
            dones_sb = bigpool.tile([12, 6, 2, BLOC], FP8, tag="dones")
            nc.sync.dma_start(out=dones_sb, in_=dones_d)
            bdec_sb = bigpool.tile([12, S // 2, 128], F16, tag="bdec")
            nc.sync.dma_start(out=bdec_sb, in_=bdec_d)

            # f16 h history; becomes u = elu(h)+1 in place, then blended.
            hist = bigpool.tile([128, S, 4, BLOC], F16, tag="hist")
            vtile = bigpool.tile([128, S // RES, 4, BLOC], F16, tag="vt")
            out_sb = bigpool.tile([128, S, 2, BLOC], F16, tag="outsb")

            dec_w = {}

            # ---------------- encoder scan ----------------
            # the whole recurrence chain runs in f16 (DVE 2x mode); the next
            # step reads hist[:, t-1] directly, and elu blocks are delayed by
            # one step so they never overwrite a slot the next step still
            # needs
            def elu_block(t0, size, ks):
                blk = hist[:, t0 : t0 + size]
                en_full = spool.tile([128, 8, 4, BLOC], F16, tag="eneg")
                en = en_full[:, :size]
                nc.vector.tensor_scalar_min(en, blk, 0.0)
                nc.vector.tensor_scalar_max(blk, blk, 0.0)
                nc.scalar.activation(out=en, in_=en, func=AF.Exp)
                nc.vector.tensor_add(blk, blk, en)
                # v-chain updates for blend positions now available:
                # v[n] = (u[4n] + v[n-1])/2
                for k in ks:
                    n = k // RES
                    if n == 0:
                        nc.vector.tensor_scalar_mul(vtile[:, 0], hist[:, 0], 0.5)
                    else:
                        nc.vector.tensor_add(vtile[:, n], hist[:, k], vtile[:, n - 1])
                        nc.vector.tensor_scalar_mul(vtile[:, n], vtile[:, n], 0.5)

            for c in range(S // ECH):
                w_sb = wpool.tile([128, ECH, 6, 16, 128], FP8, tag="w")
                nc.sync.dma_start(out=w_sb, in_=wt_d[:, c * ECH : (c + 1) * ECH])
                for i in range(ECH):
                    t = c * ECH + i
                    ps = psum.tile([128, 16, BLOC], F32, tag="ps")
                    nc.tensor.matmul(
                        ps, lhsT=benc_sb[:, t], rhs=eones_sb,
                        start=True, stop=False, skip_group_check=True,
                    )
                    # x-dependent matmuls first: the PE queue is in-order, so
                    # issuing these before the h-matmuls lets the PE work
                    # while the previous step's h is still being produced
                    for m in range(16):
                        for k in range(2):
                            nc.tensor.matmul(
                                ps[:, m], lhsT=w_sb[:, i, k, m], rhs=x_sb[:, t, k],
                                start=False, stop=(t == 0 and k == 1),
                                skip_group_check=True,
                            )
                    if t > 0:
                        # g-gate chunks (12:16) first so the tanh activation
                        # overlaps the remaining h-matmuls
                        for m in (12, 13, 14, 15, 0, 1, 2, 3, 4, 5, 6, 7, 8, 9, 10, 11):
                            for k in range(2, 6):
                                nc.tensor.matmul(
                                    ps[:, m], lhsT=w_sb[:, i, k, m], rhs=hist[:, t - 1, k - 2],
                                    start=False, stop=(k == 5),
                                    skip_group_check=True,
                                )
                    gs = gpool.tile([128, 16, BLOC], F16, tag="gs")
                    nc.scalar.activation(out=gs[:, 12:16], in_=ps[:, 12:16], func=AF.Tanh, scale=inv)
                    nc.scalar.activation(out=gs[:, 0:12], in_=ps[:, 0:12], func=AF.Sigmoid, scale=inv)
                    # c = f*h_prev + i*g ; h = o*tanh(c)
                    cc = spool.tile([128, 4, BLOC], F16, tag="cc")
                    nc.vector.tensor_mul(cc, gs[:, 0:4], gs[:, 12:16])
                    if t > 0:
                        fh = spool.tile([128, 4, BLOC], F16, tag="fh")
                        nc.vector.tensor_mul(fh, gs[:, 4:8], hist[:, t - 1])
                        nc.vector.tensor_add(cc, cc, fh)
                    tct = spool.tile([128, 4, BLOC], F16, tag="tct")
                    nc.scalar.activation(out=tct, in_=cc, func=AF.Tanh)
                    nc.vector.tensor_mul(hist[:, t], tct, gs[:, 8:12])
                    # delayed batched elu: u = relu(h) + exp(min(h,0)), the
                    # -1 is folded into the decoder bias on host
                    if t % 8 == 0 and t > 0:
                        elu_block(t - 8, 8, (t - 8, t - 4))
                    elif t == S - 4:
                        elu_block(S - 8, 4, (S - 8,))
            # finish the elu fine-grained: t=62..63 first so the decoder's
            # first pair (tsrc 63, 62) starts while 60..61 processes
            elu_block(S - 2, 2, ())
            elu_block(S - 4, 2, (S - 4,))

            # ---------------- skip blend end corrections ----------------
            # u'[4n] = v[n] + 2^-(n+1) * u[60]; descending n matches the
            # decoder's consumption order (idx 4j+3 reads t = 60-4j).
            u60 = spool.tile([128, 4, BLOC], F16, tag="u60")
            nc.vector.tensor_copy(out=u60, in_=hist[:, S - RES])
            for n in range(S // RES - 1, -1, -1):
                bc = spool.tile([128, 4, BLOC], F16, tag="bc")
                nc.vector.tensor_scalar_mul(bc, u60, 0.5 ** (n + 1))
                nc.vector.tensor_add(hist[:, n * RES], vtile[:, n], bc)

            # ---------------- decoder (parallel over idx, 2 idx/batch) ----
            rn_prev = None
            for c in range(S // DCH):
                wd_sb = wdpool.tile([128, DCH, 4, 6, 128], FP8, tag="wd")
                tail = c == 15
                if c >= 14:
                    # two partial loads into one tile: the first pair's weights
                    # land a DMA-slot earlier, so its chain overlaps the second
                    # pair's load
                    s0 = c * DCH
                    nc.sync.dma_start(out=wd_sb[:, 0:2], in_=wdt_d[:, s0 : s0 + 2])
                    nc.sync.dma_start(out=wd_sb[:, 2:4], in_=wdt_d[:, s0 + 2 : s0 + 4])
                else:
                    nc.sync.dma_start(
                        out=wd_sb, in_=wdt_d[:, c * DCH : (c + 1) * DCH]
                    )
                # chunk-batched act path (c<15): one tanh(c) and one output
                # tanh per 4-idx chunk instead of per pair — ~370ns less Act
                # busy per chunk, relieving the Act backlog that gates the
                # decoder tail. The final chunk keeps the per-pair path for
                # minimum latency after the last weight chunk lands.
                gds = []
                tail_hd = []
                if not tail:
                    ccd = spool.tile([128, 2, 2, 2, BLOC], F16, tag="ccd")
                    crn = spool.tile([128, 4, 2, BLOC], F16, tag="crn")
                for jp in range(DCH // 2):
                    i0 = c * DCH + jp * 2  # idx pair (i0, i0+1)
                    psd = psum.tile([128, 6, 2, BLOC], F32, tag="psd")
                    nc.tensor.matmul(
                        psd, lhsT=bdec_sb[:, i0 // 2], rhs=dones_sb,
                        start=True, stop=False, skip_group_check=True,
                    )
                    for j in range(2):
                        tsrc = S - 1 - (i0 + j)
                        for m in range(6):
                            for k in range(4):
                                nc.tensor.matmul(
                                    psd[:, m, j],
                                    lhsT=wd_sb[:, jp * 2 + j, k, m],
                                    rhs=hist[:, tsrc, k],
                                    start=False, stop=(k == 3),
                                    skip_group_check=True,
                                )
                    gd = gpool.tile([128, 6, 2, BLOC], F16, tag="gd")
                    gds.append(gd)
                    # g rows are host-scaled by 2, so one sigmoid covers all 6
                    # chunks; tanh(g) = 2*sig(2g) - 1 via a fused DVE affine.
                    # One Act op instead of two relieves the Act backlog that
                    # gates the decoder tail.
                    nc.scalar.activation(out=gd, in_=psd, func=AF.Sigmoid, scale=inv)
                    nc.vector.tensor_scalar(
                        gd[:, 4:6], gd[:, 4:6], 2.0, -1.0,
                        op0=mybir.AluOpType.mult, op1=mybir.AluOpType.add,
                    )
                    if tail:
                        # per-pair hd chain only; rn/out-tanh of pair 0 is
                        # emitted before pair 1's, but pair 0's OUT tanh is
                        # deferred below so the Act queue order becomes
                        # [sig0, tanhcd0, sig1, tanhcd1, out0, out1] — pair
                        # 1's sigmoid is not stuck behind pair 0's out tanh
                        cd = spool.tile([128, 2, 2, BLOC], F16, tag="cd")
                        nc.vector.tensor_mul(cd, gd[:, 0:2], gd[:, 4:6])
                        nc.scalar.activation(out=cd, in_=cd, func=AF.Tanh)
                        nc.vector.tensor_mul(cd, cd, gd[:, 2:4])  # hd
                        hdT = cd.transpose([0, 2, 1, 3])  # [128, j, hh, b]
                        tail_hd.append(hdT)
                        if jp == 0:
                            rn = spool.tile([128, 2, 2, BLOC], F16, tag="rn")
                            nc.vector.tensor_copy(out=rn[:, 0], in_=hdT[:, 0])
                            nc.vector.tensor_add(rn[:, 1], rn[:, 0], hdT[:, 1])
                            tail_rn = rn
                    else:
                        nc.vector.tensor_mul(ccd[:, jp], gd[:, 0:2], gd[:, 4:6])
                if tail:
                    nc.scalar.activation(out=out_sb[:, 60:62], in_=tail_rn, func=AF.Tanh)
                    nc.gpsimd.dma_start(out=out_d[:, 60:62], in_=out_sb[:, 60:62])
                    rn2 = spool.tile([128, 2, 2, BLOC], F16, tag="rn")
                    nc.vector.tensor_add(rn2[:, 0], tail_rn[:, 1], tail_hd[1][:, 0])
                    nc.vector.tensor_add(rn2[:, 1], rn2[:, 0], tail_hd[1][:, 1])
                    nc.scalar.activation(out=out_sb[:, 62:64], in_=rn2, func=AF.Tanh)
                    nc.sync.dma_start(out=out_d[:, 62:64], in_=out_sb[:, 62:64])
                else:
                    nc.scalar.activation(out=ccd, in_=ccd, func=AF.Tanh)
                    for jp in range(2):
                        nc.vector.tensor_mul(ccd[:, jp], ccd[:, jp], gds[jp][:, 2:4])
                    # cumsum within the RES-block (== this chunk) and one
                    # batched output tanh
                    hdT0 = ccd[:, 0].transpose([0, 2, 1, 3])
                    hdT1 = ccd[:, 1].transpose([0, 2, 1, 3])
                    nc.vector.tensor_copy(out=crn[:, 0], in_=hdT0[:, 0])
                    nc.vector.tensor_add(crn[:, 1], crn[:, 0], hdT0[:, 1])
                    nc.vector.tensor_add(crn[:, 2], crn[:, 1], hdT1[:, 0])
                    nc.vector.tensor_add(crn[:, 3], crn[:, 2], hdT1[:, 1])
                    nc.scalar.activation(
                        out=out_sb[:, c * DCH : (c + 1) * DCH], in_=crn, func=AF.Tanh
                    )
                    # out DMA per 4-step chunk on the gpsimd SWDGE queue:
                    # keeps the SP weight stream free of head-of-line blocking
                    # on decoder compute, and the DMA device fills its idle
                    # slots with these (the tail chunk's outs are inline above)
                    s0 = c * DCH
                    nc.gpsimd.dma_start(
                        out=out_d[:, s0 : s0 + DCH], in_=out_sb[:, s0 : s0 + DCH]
                    )
    nc.finalize()
    return nc


def _host_prep(inputs):
    f16 = np.float16
    f8 = ml_dtypes.float8_e3m4
    # encoder: gate order [i, f, o, g]
    eperm = np.r_[0:512, 512:1024, 1536:2048, 1024:1536]
    W_all = np.concatenate([inputs["Wih_enc"], inputs["Whh_enc"]], axis=2)[:, eperm, :] * WSCALE
    # [t, 16m, 128q, 6k, 128p] -> [p, t, k, m, q]
    wt = np.ascontiguousarray(
        W_all.reshape(S, 16, 128, 6, 128).transpose(4, 0, 3, 1, 2)
    ).astype(f8)
    benc = np.ascontiguousarray(
        ((inputs["bih_enc"] + inputs["bhh_enc"])[:, eperm] * WSCALE)
        .reshape(S, 16, 128)
        .transpose(1, 0, 2)
    ).astype(f8)
    eones = np.ascontiguousarray(
        np.repeat(np.eye(16, dtype=np.float32)[:, :, None], BLOC, axis=2)
    ).astype(f8)
    # decoder: gate order [i, o, g]; g rows scaled by 2 so the device can use
    # a single sigmoid table: tanh(g) = 2*sigmoid(2g) - 1
    dperm = np.r_[0:256, 768:1024, 512:768]
    Wd = inputs["Wih_dec"][:, dperm, :] * WSCALE
    Wd[:, 512:768, :] *= 2.0
    wd8 = np.ascontiguousarray(
        Wd.reshape(S, 6, 128, 4, 128).transpose(4, 0, 3, 1, 2)  # [p,t,k,m,q]
    ).astype(f8)
    # fold elu's "-1" into the bias: subtract row sums of the quantized W
    corr = wd8.astype(np.float32).sum(axis=(0, 2))  # [t, m, q]
    braw = (inputs["bih_dec"] + inputs["bhh_dec"])[:, dperm] * WSCALE
    braw[:, 512:768] *= 2.0  # match the g-row scaling
    bd = braw.reshape(S, 6, 128) - corr
    # idx-pair packing: bdec[(m*2+j), pair, q] = bd[2*pair+j, m, q]
    bdec = np.ascontiguousarray(
        bd.reshape(S // 2, 2, 6, 128).transpose(2, 1, 0, 3).reshape(12, S // 2, 128)
    ).astype(f16)
    dones = np.ascontiguousarray(
        np.repeat(
            np.eye(12, dtype=np.float32).reshape(12, 6, 2)[:, :, :, None], BLOC, axis=3
        )
    ).astype(f8)
    # x ships as fp8 e3m4: 1.8% elementwise quantization, which adds ~sqrt(2)
    # on the Wih*x error term only (the same term already carries the weight
    # quantization error) — measured rel err stays under the 2e-2 gate, and
    # the halved x bytes shave 1.46us off the DMA floor
    xr = np.ascontiguousarray(
        inputs["x"].reshape(B, 2, 128, S).transpose(2, 3, 1, 0)
    ).astype(f8)
    return wt, benc, eones, wd8, bdec, dones, xr


def kernel(**inputs):
    inputs = {k: np.asarray(v) for k, v in inputs.items()}
    if "nc" not in _STATE:
        _STATE["nc"] = _build_module()
    nc = _STATE["nc"]
    wt, benc, eones, wdt, bdec, dones, xr = _host_prep(inputs)
    in_maps = []
    for c in range(NCORES):
        in_maps.append(
            {
                "wt": wt,
                "wdt": wdt,
                "benc": benc,
                "bdec": bdec,
                "eones": eones,
                "dones": dones,
                "xr": np.ascontiguousarray(xr[:, :, :, c * BLOC : (c + 1) * BLOC]),
            }
        )
    res = bass_utils.run_bass_kernel_spmd(
        nc,
        in_maps,
        core_ids=list(range(NCORES)),
        trace=bool(int(os.environ.get("BASS_KERNEL_TRACE", "0"))),
    )
    _STATE["last_results"] = res
    outs = []
    for c in range(NCORES):
        o = np.asarray(res.results[c]["out"]).astype(np.float32)  # [128, S, 2, BLOC]
        outs.append(
            np.ascontiguousarray(
                o.transpose(3, 2, 0, 1).reshape(BLOC, 2 * 128, S)[:, :, ::-1]
            )
        )
    return np.concatenate(outs, axis=0).astype(np.float32)

